# revision 7
# baseline (speedup 1.0000x reference)
"""Trainium2 Bass kernel for CartNN minimal-NEAT forward pass.

Computes out = tanh(tanh(x @ w + b))[:, None] for x [16384, 4096] f32,
w [4096] f32, b [1] f32, data-parallel across 8 NeuronCores (2048 batch
rows per core). Memory-bound: past the f32 roofline the only lever is
traffic, so the host casts x/w to fp16 (rel err 1.8e-3 vs the 2e-2
gate) and each core streams 16 MiB at the ~407 GB/s descriptor-limited
HBM rate (16-32 KiB/partition contiguous descriptors via a host-packed
K-major chunk layout).

v3: TensorE matvec with K on partitions + PE column tiling. The DVE
fused mul+reduce only has a 1x uop (4.4 us/[128,4096] tile) which put
a ~71 us floor on a DVE kernel; the PE streams 1 col/cycle @ 2.4 GHz
warm (216 ns per 512-col matmul, measured). Host layout:

    H[k', c*2048 + n] = x[n, 128c + k']   (k' partition, c chunk, n batch)

Per chunk c, 4 accumulating matmuls (one per 512-col batch block)
    psum_blk += wT[:, c:c+1].T @ H[:, c*2048+512b : ...]
with each block PINNED to its own 32-wide PE column-group
(tile_position=(0, 32b), psum row 32b) so the 4 matmuls of a chunk run
CONCURRENTLY in the array - the post-stream PE tail drops ~4.5 -> ~1.5
us, and the psum lives on 4 partition rows so each tanh pass is one
wide-lane ACT op over [97, 512] (~0.9 us vs 2 us serial). b is folded
into the accumulation by a tiny K=1 matmul per block (b fp16
stationary, ones[1,512] moving). The 4 output slices DMA out on 4
idle rings (sync/vector/gpsimd/tensor) in parallel.
"""

import numpy as np

import concourse.bacc as bacc
import concourse.mybir as mybir
from concourse.bass_utils import run_bass_kernel_spmd
from concourse.tile import TileContext

N_CORES = 8
BATCH = 16384
IN_SIZE = 4096
P = 128
B_PER_CORE = BATCH // N_CORES  # 2048
N_CHUNKS = IN_SIZE // P  # 32 K-chunks of 128
N_BLOCKS = B_PER_CORE // 512  # 4 psum blocks of 512 batch cols
FREE = N_CHUNKS * B_PER_CORE  # 65536 fp16 elems per partition
SUB_CHUNKS = (8, 8, 8, 4, 4)  # sub-DMA sizes; 8 chunks = 32 KiB/partition

_NC_CACHE = None


def _build():
    nc = bacc.Bacc(
        "TRN2",
        target_bir_lowering=False,
        debug=False,
        num_devices=N_CORES,
    )
    x = nc.dram_tensor("x", [P, FREE], mybir.dt.float16, kind="ExternalInput")
    w = nc.dram_tensor("w", [P, N_CHUNKS], mybir.dt.float16, kind="ExternalInput")
    b = nc.dram_tensor("b", [1], mybir.dt.float16, kind="ExternalInput")
    y = nc.dram_tensor("y", [B_PER_CORE, 1], mybir.dt.float32, kind="ExternalOutput")
    yT = y.rearrange("(a n) o -> a (n o)", a=1)  # [1, 2048] contiguous

    with TileContext(nc) as tc:
        with (
            tc.tile_pool(name="xpool", bufs=1) as xpool,
            tc.tile_pool(name="consts", bufs=1) as cpool,
            tc.tile_pool(name="psum", bufs=1, space="PSUM") as ppool,
        ):
            # x sub-DMAs first on the (otherwise untouched) sync ring.
            X = xpool.tile([P, FREE], mybir.dt.float16)
            off = 0
            for nch in SUB_CHUNKS:
                seg = slice(off * B_PER_CORE, (off + nch) * B_PER_CORE)
                nc.sync.dma_start(out=X[:, seg], in_=x[:, seg])
                off += nch

            # w (pre-transposed [128, 32] on host) + b on the gpsimd ring
            # (idle; keeps both sync and scalar sequencers clear).
            wT = cpool.tile([P, N_CHUNKS], mybir.dt.float16)
            nc.gpsimd.dma_start(out=wT[:], in_=w[:, :])
            b_11 = cpool.tile([1, 1], mybir.dt.float16)
            nc.gpsimd.dma_start(out=b_11[:], in_=b[None, :])
            ones_512 = cpool.tile([1, 512], mybir.dt.float16)
            nc.vector.memset(ones_512[:], 1.0)

            # PE matvec: block b accumulates in psum row 32b, column group
            # b of the PE array. The 4 blocks of a chunk run concurrently.
            psum = ppool.tile([97, 512], mybir.dt.float32)
            for c in range(N_CHUNKS):
                for blk in range(N_BLOCKS):
                    cs = slice(
                        c * B_PER_CORE + blk * 512, c * B_PER_CORE + (blk + 1) * 512
                    )
                    r = 32 * blk
                    nc.tensor.matmul(
                        psum[r : r + 1, :],
                        wT[:, c : c + 1],
                        X[:, cs],
                        start=(c == 0),
                        stop=False,
                        tile_position=(0, r),
                        skip_group_check=True,
                    )
            # Fold the bias into each block's sum: psum_blk += b * ones.
            for blk in range(N_BLOCKS):
                r = 32 * blk
                nc.tensor.matmul(
                    psum[r : r + 1, :],
                    b_11[:],
                    ones_512[:],
                    start=False,
                    stop=True,
                    tile_position=(0, r),
                    skip_group_check=True,
                )

            # tanh(tanh(psum)) - each pass is ONE wide-lane ACT op (rows
            # 0/32/64/96 are parallel lanes; free dim only 512).
            h_sb = cpool.tile([97, 512], mybir.dt.float32)
            nc.scalar.activation(
                h_sb[:], psum[:], mybir.ActivationFunctionType.Tanh
            )
            y_sb = cpool.tile([97, 512], mybir.dt.float32)
            nc.scalar.activation(
                y_sb[:], h_sb[:], mybir.ActivationFunctionType.Tanh
            )

            # Output on the 3 DMA-capable rings in parallel: blocks 0+3
            # as one strided-AP DMA on sync, block 1 on scalar, block 2
            # on gpsimd.
            yG = y.rearrange("(g n) o -> g (n o)", g=N_BLOCKS)  # [4, 512]
            nc.sync.dma_start(out=yG[0::3], in_=y_sb[0::96, :])
            nc.scalar.dma_start(out=yG[1:2], in_=y_sb[32:33, :])
            nc.gpsimd.dma_start(out=yG[2:3], in_=y_sb[64:65, :])
    nc.compile()
    return nc


def _get_nc():
    global _NC_CACHE
    if _NC_CACHE is None:
        _NC_CACHE = _build()
    return _NC_CACHE


def _pack_x(xs):
    """[2048, 4096] f32 -> [128, 65536] fp16, H[k', c*2048+n] = xs[n, 128c+k']."""
    xt = xs.T.astype(np.float16)  # [4096, 2048]
    # [32, 128, 2048] -> [128, 32, 2048]
    return np.ascontiguousarray(
        xt.reshape(N_CHUNKS, P, B_PER_CORE).transpose(1, 0, 2)
    ).reshape(P, FREE)


def _run(x, w, b, **spmd_kwargs):
    """Shard, execute on 8 cores, gather. Returns (out, BassKernelResults)."""
    x = np.asarray(x, dtype=np.float32)
    assert x.shape == (BATCH, IN_SIZE), x.shape
    w16 = np.asarray(w, dtype=np.float32).astype(np.float16)
    wT = np.ascontiguousarray(w16.reshape(N_CHUNKS, P).T)  # [128, 32]
    b16 = np.asarray(b, dtype=np.float32).astype(np.float16).reshape(1)

    nc = _get_nc()
    in_maps = [
        {
            "x": _pack_x(x[c * B_PER_CORE : (c + 1) * B_PER_CORE]),
            "w": wT,
            "b": b16,
        }
        for c in range(N_CORES)
    ]
    res = run_bass_kernel_spmd(nc, in_maps, list(range(N_CORES)), **spmd_kwargs)
    out = np.concatenate(
        [np.asarray(res.results[c]["y"]) for c in range(N_CORES)], axis=0
    )
    return out.astype(np.float32, copy=False), res


def kernel(x, w, b):
    try:
        out, _ = _run(x, w, b)
    except Exception:
        # Transient device-wedge (NRT_EXEC_UNIT_UNRECOVERABLE) has been
        # observed once on a first run and succeeded on retry.
        out, _ = _run(x, w, b)
    return out


if __name__ == "__main__":
    rng = np.random.default_rng(0)
    x = rng.standard_normal((BATCH, IN_SIZE), dtype=np.float32)
    w = rng.standard_normal(IN_SIZE, dtype=np.float32)
    b = rng.standard_normal(1).astype(np.float32)
    out = kernel(x, w, b)
    ref = np.tanh(np.tanh(x @ w + b[0]))[:, None]
    err = np.linalg.norm(out - ref) / np.linalg.norm(ref)
    print("rel err:", err)
